# revision 1
# baseline (speedup 1.0000x reference)
"""Trainium2 Bass kernel for nn_AttentionBlock (GroupNorm -> QKV -> 8-head
attention over T=1024 -> proj -> residual) on x[8, 512, 32, 32] f32.

Sharding: data-parallel over batch: core b handles sample b. No collectives.

Per-core layout strategy:
  - x, xn: [C=512, T=1024] as 4 tiles [128, 1024] (channels on partitions).
  - GroupNorm stats (32 groups of 16ch x 1024): group sums of x and x^2 via
    PE matmuls with 0/1 indicator weights; mean/rstd broadcast back to
    channel partitions via tiny indicator-transpose matmuls.
  - QKV with host-pre-transposed weights. q,k produced in [head-pair]
    m-tiles: tile 2p = [q_{2p} | q_{2p+1}], tile 2p+1 = [k_{2p} | k_{2p+1}]
    so each head's q and k share a partition range (0:64 or 64:128).
  - v produced TRANSPOSED (vT[t, c']) directly by swapping matmul operands,
    so the attention's second matmul needs no on-chip transposes at all.
  - Scores computed transposed: S^T = k^T q -> [s, t] with softmax dim s on
    partitions. exp on ScalarE (scale=1/8 folded in, no max-subtraction
    needed; |scores| ~ N(0,1)). Softmax denominators via ones-vector
    matmuls packed 4-way into PE column groups; division via
    reciprocal_approx_fast + partition-broadcast DMA, applied during the
    h-eviction multiply.
  - proj with residual fused into PSUM eviction; biases folded into
    matmuls as K=1 rank-1 updates where they are free-dim indexed.

All matmul operands are bf16 (1 cycle/row on TRN2 vs 4 for float32);
PSUM accumulation is f32.
"""

import numpy as np

import concourse.bacc as bacc
import concourse.bass as bass
import concourse.mybir as mybir
import concourse.tile as tile
from concourse.bass_utils import run_bass_kernel_spmd

F32 = mybir.dt.float32
BF16 = mybir.dt.bfloat16
AF = mybir.ActivationFunctionType
ALU = mybir.AluOpType

B, C, H, W = 8, 512, 32, 32
T = H * W  # 1024
NH = 8  # heads
CH = C // NH  # 64 head channels
GROUPS = 32
GSIZE = C // GROUPS  # 16 channels per group
EPS = 1e-5
NCT = C // 128  # 4 channel tiles
NST = T // 128  # 8 spatial tiles
NPAIR = NH // 2  # 4 head pairs


def _bcast_ap(src, n):
    """Partition-broadcast AP: replicate src's single partition n times."""
    ap = [[0, n]] + [list(d) for d in list(src.ap)[1:]]
    return bass.AP(tensor=src.tensor, offset=src.offset, ap=ap)


def build_nc(debug_taps=False):
    nc = bacc.Bacc(
        "TRN2",
        target_bir_lowering=False,
        debug=False,
        enable_asserts=False,
        num_devices=8,
    )

    x_d = nc.dram_tensor("x", [C, T], F32, kind="ExternalInput")
    wqk_d = nc.dram_tensor("wqk", [C, 2 * C], BF16, kind="ExternalInput")
    wv_d = nc.dram_tensor("wv", [C, C], BF16, kind="ExternalInput")
    wp_d = nc.dram_tensor("wp", [C, C], BF16, kind="ExternalInput")
    bqk_d = nc.dram_tensor("bqk", [8, 128], F32, kind="ExternalInput")
    bv_d = nc.dram_tensor("bv", [1, C], BF16, kind="ExternalInput")
    bp_d = nc.dram_tensor("bp", [1, C], BF16, kind="ExternalInput")
    gs_d = nc.dram_tensor("gs", [NCT, 128], F32, kind="ExternalInput")
    gb_d = nc.dram_tensor("gb", [NCT, 128], F32, kind="ExternalInput")
    ind_d = nc.dram_tensor("ind", [NCT, 128, GROUPS], BF16, kind="ExternalInput")
    indt_d = nc.dram_tensor("indt", [NCT, GROUPS, 128], F32, kind="ExternalInput")
    out_d = nc.dram_tensor("out", [C, T], F32, kind="ExternalOutput")
    dbg = {}
    if debug_taps:
        dbg["xn"] = nc.dram_tensor("dbg_xn", [NCT, 128, T], BF16, kind="ExternalOutput")
        dbg["qk"] = nc.dram_tensor("dbg_qk", [8, 128, T], BF16, kind="ExternalOutput")
        dbg["vt"] = nc.dram_tensor("dbg_vt", [NST, 128, C], BF16, kind="ExternalOutput")
        dbg["es"] = nc.dram_tensor("dbg_es", [2, 128, T], BF16, kind="ExternalOutput")
        dbg["invs"] = nc.dram_tensor("dbg_invs", [128, 512], F32, kind="ExternalOutput")
        dbg["inv"] = nc.dram_tensor("dbg_inv", [128, T], F32, kind="ExternalOutput")
        dbg["ha"] = nc.dram_tensor("dbg_ha", [NCT, 128, T], BF16, kind="ExternalOutput")

    CS = [slice(0, 512), slice(512, 1024)]

    with tile.TileContext(nc) as tc:
        with (
            tc.tile_pool(name="sb", bufs=1) as sb,
            tc.tile_pool(name="ps", bufs=1, space="PSUM") as ps,
        ):
            def big():
                t = ps.tile([128, 512], F32, tag="big", bufs=5, name="bigps")
                return t

            # ---- input loads: GN-critical first, wp last --------------------
            xs, xns, wqks, wvs, wps = [], [], [], [], []
            inds, indts, gss, gbs = [], [], [], []
            for i in range(NCT):
                xt = sb.tile([128, T], F32, tag="x", bufs=NCT, name=f"x{i}")
                nc.sync.dma_start(out=xt, in_=x_d.ap()[128 * i : 128 * (i + 1), :])
                xs.append(xt)
                it = sb.tile([128, GROUPS], BF16, tag="ind", bufs=NCT, name=f"ind{i}")
                nc.sync.dma_start(out=it, in_=ind_d.ap()[i, :, :])
                inds.append(it)
                itt = sb.tile([GROUPS, 128], F32, tag="indt", bufs=NCT, name=f"indt{i}")
                nc.sync.dma_start(out=itt, in_=indt_d.ap()[i, :, :])
                indts.append(itt)
                gst = sb.tile([128, 1], F32, tag="gs", bufs=NCT, name=f"gs{i}")
                nc.sync.dma_start(out=gst, in_=gs_d.ap()[i, :].unsqueeze(1))
                gss.append(gst)
                gbt = sb.tile([128, 1], F32, tag="gb", bufs=NCT, name=f"gb{i}")
                nc.sync.dma_start(out=gbt, in_=gb_d.ap()[i, :].unsqueeze(1))
                gbs.append(gbt)
            for i in range(NCT):
                wq = sb.tile([128, 2 * C], BF16, tag="wqk", bufs=NCT, name=f"wqk{i}")
                nc.sync.dma_start(out=wq, in_=wqk_d.ap()[128 * i : 128 * (i + 1), :])
                wqks.append(wq)
            for i in range(NCT):
                wv = sb.tile([128, C], BF16, tag="wv", bufs=NCT, name=f"wv{i}")
                nc.sync.dma_start(out=wv, in_=wv_d.ap()[128 * i : 128 * (i + 1), :])
                wvs.append(wv)
            bqks = []
            for mt in range(8):
                bq = sb.tile([128, 1], F32, tag="bqk", bufs=8, name=f"bqk{mt}")
                nc.sync.dma_start(out=bq, in_=bqk_d.ap()[mt, :].unsqueeze(1))
                bqks.append(bq)
            bv_row = sb.tile([1, C], BF16, tag="bv", name="bv_row")
            nc.sync.dma_start(out=bv_row, in_=bv_d.ap())
            for i in range(NCT):
                wp = sb.tile([128, C], BF16, tag="wp", bufs=NCT, name=f"wp{i}")
                nc.sync.dma_start(out=wp, in_=wp_d.ap()[128 * i : 128 * (i + 1), :])
                wps.append(wp)
            bp_row = sb.tile([1, C], BF16, tag="bp", name="bp_row")
            nc.sync.dma_start(out=bp_row, in_=bp_d.ap())

            ones_col = sb.tile([128, 1], BF16, tag="ones_c", name="ones_col")
            nc.vector.memset(ones_col, 1.0)
            ones_row = sb.tile([1, T], BF16, tag="ones_r", name="ones_row")
            nc.vector.memset(ones_row, 1.0)
            zeros_row = sb.tile([1, 512], BF16, tag="zeros_r", name="zeros_row")
            nc.vector.memset(zeros_row, 0.0)

            # ---- GroupNorm statistics ---------------------------------------
            s1c = [big() for _ in range(2)]
            s2c = [big() for _ in range(2)]
            for i in range(NCT):
                xb = sb.tile([128, T], BF16, tag="xb", bufs=2, name=f"xb{i}")
                nc.vector.tensor_copy(xb, xs[i])
                xsq = sb.tile([128, T], BF16, tag="xsq", bufs=2, name=f"xsq{i}")
                nc.vector.tensor_mul(xsq, xs[i], xs[i])
                first, last = i == 0, i == NCT - 1
                for n in range(2):
                    nc.tensor.matmul(
                        out=s1c[n][0:GROUPS, :], lhsT=inds[i],
                        rhs=xb[:, CS[n]], start=first, stop=last,
                    )
                    nc.tensor.matmul(
                        out=s2c[n][0:GROUPS, :], lhsT=inds[i],
                        rhs=xsq[:, CS[n]], start=first, stop=last,
                    )

            def chunk_reduce(chunks, name):
                ra = sb.tile([GROUPS, 2], F32, tag="gnsm", bufs=8, name=f"{name}a")
                for n in range(2):
                    nc.vector.tensor_reduce(
                        out=ra[:, n : n + 1], in_=chunks[n][0:GROUPS, :],
                        axis=mybir.AxisListType.X, op=ALU.add,
                    )
                rt = sb.tile([GROUPS, 1], F32, tag="gnsm", bufs=8, name=f"{name}t")
                nc.vector.tensor_add(rt, ra[:, 0:1], ra[:, 1:2])
                return rt

            red1 = chunk_reduce(s1c, "red1")
            red2 = chunk_reduce(s2c, "red2")
            inv_n = 1.0 / (GSIZE * T)
            mr32 = sb.tile([GROUPS, 2], F32, tag="gnsm", bufs=8, name="mr32")
            nc.vector.tensor_scalar_mul(out=mr32[:, 0:1], in0=red1, scalar1=inv_n)
            ex2 = sb.tile([GROUPS, 1], F32, tag="gnsm", bufs=8, name="ex2")
            nc.vector.tensor_scalar_mul(out=ex2, in0=red2, scalar1=inv_n)
            msq = sb.tile([GROUPS, 1], F32, tag="gnsm", bufs=8, name="msq")
            nc.vector.tensor_mul(msq, mr32[:, 0:1], mr32[:, 0:1])
            var = sb.tile([GROUPS, 1], F32, tag="gnsm", bufs=8, name="var")
            nc.vector.tensor_sub(var, ex2, msq)
            eps_t = sb.tile([GROUPS, 1], F32, tag="gnsm", bufs=8, name="eps_t")
            nc.vector.memset(eps_t, EPS)
            lnv = sb.tile([GROUPS, 1], F32, tag="gnsm", bufs=8, name="lnv")
            nc.scalar.activation(out=lnv, in_=var, func=AF.Ln, bias=eps_t, scale=1.0)
            nc.scalar.activation(out=mr32[:, 1:2], in_=lnv, func=AF.Exp, scale=-0.5)

            for i in range(NCT):
                mrb = big()
                nc.tensor.matmul(out=mrb[:, 0:2], lhsT=indts[i], rhs=mr32)
                a_t = sb.tile([128, 1], F32, tag="gA", bufs=NCT, name=f"gA{i}")
                nc.vector.tensor_mul(a_t, mrb[:, 1:2], gss[i])
                tmp = sb.tile([128, 1], F32, tag="gT", bufs=2, name=f"gT{i}")
                nc.vector.tensor_mul(tmp, mrb[:, 0:1], a_t)
                b_t = sb.tile([128, 1], F32, tag="gB", bufs=NCT, name=f"gB{i}")
                nc.vector.tensor_sub(b_t, gbs[i], tmp)
                xn = sb.tile([128, T], BF16, tag="xn", bufs=NCT, name=f"xn{i}")
                nc.vector.tensor_scalar(
                    out=xn, in0=xs[i], scalar1=a_t, scalar2=b_t,
                    op0=ALU.mult, op1=ALU.add,
                )
                xns.append(xn)
                if debug_taps:
                    nc.sync.dma_start(out=dbg["xn"].ap()[i], in_=xn)

            # ---- QKV ---------------------------------------------------------
            qks = [None] * 8
            vts = [None] * NST

            def emit_qk(mt):
                qk = sb.tile([128, T], BF16, tag="qk", bufs=8, name=f"qk{mt}")
                for n in range(2):
                    qp = big()
                    for i in range(NCT):
                        nc.tensor.matmul(
                            out=qp,
                            lhsT=wqks[i][:, 128 * mt : 128 * (mt + 1)],
                            rhs=xns[i][:, CS[n]],
                            start=(i == 0),
                            stop=(i == NCT - 1),
                        )
                    nc.vector.tensor_scalar_add(
                        out=qk[:, CS[n]], in0=qp, scalar1=bqks[mt]
                    )
                qks[mt] = qk
                if debug_taps:
                    nc.sync.dma_start(out=dbg["qk"].ap()[mt], in_=qk)

            def emit_vt(st):
                vp = big()
                for i in range(NCT):
                    nc.tensor.matmul(
                        out=vp,
                        lhsT=xns[i][:, 128 * st : 128 * (st + 1)],
                        rhs=wvs[i],
                        start=(i == 0),
                        stop=False,
                    )
                nc.tensor.matmul(
                    out=vp, lhsT=ones_row[0:1, 0:128], rhs=bv_row,
                    start=False, stop=True,
                )
                vt = sb.tile([128, C], BF16, tag="vt", bufs=NST, name=f"vt{st}")
                nc.vector.tensor_copy(vt, vp)
                vts[st] = vt
                if debug_taps:
                    nc.sync.dma_start(out=dbg["vt"].ap()[st], in_=vt)

            emit_qk(0)
            emit_qk(1)
            for st in range(NST):
                emit_vt(st)

            # ---- attention ---------------------------------------------------
            has = []
            for p in range(NPAIR):
                qtile, ktile = qks[2 * p], qks[2 * p + 1]
                hc = [
                    ps.tile([128, 512], F32, tag="h", bufs=3, name=f"hc{p}_{n}")
                    for n in range(2)
                ]
                sums = ps.tile([128, 512], F32, tag="h", bufs=3, name=f"sums{p}")
                # open one accumulation group per shared bank (zeroing rank-1
                # matmul with start=True); packed matmuls then use start=False.
                for bank in (hc[0], hc[1], sums):
                    nc.tensor.matmul(
                        out=bank, lhsT=zeros_row[0:1, 0:128], rhs=zeros_row,
                        start=True, stop=False,
                    )

                # software pipeline over s-tiles; all PSUM tiles are single
                # chunks [128, 512]; exp per chunk so slots free early.
                sts, ests = {}, {}

                def st_mms(st):
                    ss = slice(128 * st, 128 * (st + 1))
                    tiles = []
                    for n in range(2):
                        se = big()
                        so = big()
                        nc.tensor.matmul(
                            out=se, lhsT=ktile[0:64, ss], rhs=qtile[0:64, CS[n]],
                        )
                        nc.tensor.matmul(
                            out=so, lhsT=ktile[64:128, ss], rhs=qtile[64:128, CS[n]],
                        )
                        tiles.append((se, so))
                    sts[st] = tiles

                def exps(st):
                    es = []
                    for n in range(2):
                        se, so = sts[st][n]
                        ee = sb.tile([128, 512], BF16, tag="es", bufs=12,
                                     name=f"ee{p}_{st}_{n}")
                        eo = sb.tile([128, 512], BF16, tag="es", bufs=12,
                                     name=f"eo{p}_{st}_{n}")
                        nc.scalar.activation(out=ee, in_=se, func=AF.Exp, scale=0.125)
                        nc.scalar.activation(out=eo, in_=so, func=AF.Exp, scale=0.125)
                        es.append((ee, eo))
                        if debug_taps and p == 0 and st == 0:
                            nc.sync.dma_start(out=dbg["es"].ap()[0][:, CS[n]], in_=ee)
                            nc.sync.dma_start(out=dbg["es"].ap()[1][:, CS[n]], in_=eo)
                    ests[st] = es

                def h_sums_mms(st):
                    for n in range(2):
                        ee, eo = ests[st][n]
                        nc.tensor.matmul(
                            out=hc[n][0:64, :],
                            lhsT=vts[st][:, 128 * p : 128 * p + 64],
                            rhs=ee, start=False, stop=False,
                        )
                        nc.tensor.matmul(
                            out=hc[n][64:128, :],
                            lhsT=vts[st][:, 128 * p + 64 : 128 * p + 128],
                            rhs=eo, start=False, stop=False,
                        )
                    for n in range(2):
                        ee, eo = ests[st][n]
                        nc.tensor.matmul(
                            out=sums[32 * n : 32 * n + 1, :], lhsT=ones_col,
                            rhs=ee, start=False, stop=False,
                            tile_position=(0, 32 * n),
                        )
                        nc.tensor.matmul(
                            out=sums[64 + 32 * n : 64 + 32 * n + 1, :],
                            lhsT=ones_col,
                            rhs=eo, start=False, stop=False,
                            tile_position=(0, 64 + 32 * n),
                        )

                st_mms(0)
                for st in range(NST):
                    exps(st)
                    if st + 1 < NST:
                        st_mms(st + 1)
                    h_sums_mms(st)

                # close each shared-bank accumulation group (rank-1, adds 0).
                for bank in (hc[0], hc[1], sums):
                    nc.tensor.matmul(
                        out=bank[:, 0:1], lhsT=zeros_row[0:1, 0:128],
                        rhs=zeros_row[0:1, 0:1], start=False, stop=True,
                    )
                invs = sb.tile([128, 512], F32, tag="invs", bufs=2, name=f"invs{p}")
                nc.vector.reciprocal_approx_fast(out=invs, in_=sums)
                if debug_taps and p == 0:
                    nc.sync.dma_start(out=dbg["invs"].ap(), in_=invs)
                inv = sb.tile([128, T], F32, tag="inv", bufs=2, name=f"inv{p}")
                invd = nc.dram_tensor(f"invd{p}", [4, 512], F32)
                for j, row in enumerate((0, 32, 64, 96)):
                    nc.sync.dma_start(
                        out=invd.ap()[j : j + 1, :], in_=invs[row : row + 1, :]
                    )
                for j, (dp, n) in enumerate(((0, 0), (0, 1), (64, 0), (64, 1))):
                    nc.sync.dma_start(
                        out=inv[dp : dp + 64, CS[n]],
                        in_=_bcast_ap(invd.ap()[j : j + 1, :], 64),
                    )
                ha = sb.tile([128, T], BF16, tag="ha", bufs=NPAIR, name=f"ha{p}")
                for n in range(2):
                    nc.vector.tensor_mul(ha[:, CS[n]], hc[n], inv[:, CS[n]])
                has.append(ha)
                if debug_taps:
                    nc.sync.dma_start(out=dbg["ha"].ap()[p], in_=ha)
                    if p == 0:
                        nc.sync.dma_start(out=dbg["inv"].ap(), in_=inv)
                if p < NPAIR - 1:
                    emit_qk(2 * p + 2)
                    emit_qk(2 * p + 3)

            # ---- proj + residual --------------------------------------------
            for mt in range(NCT):
                ot = sb.tile([128, T], F32, tag="ot", bufs=2, name=f"ot{mt}")
                for n in range(2):
                    pp = big()
                    for i in range(NCT):
                        nc.tensor.matmul(
                            out=pp,
                            lhsT=wps[i][:, 128 * mt : 128 * (mt + 1)],
                            rhs=has[i][:, CS[n]],
                            start=(i == 0),
                            stop=False,
                        )
                    nc.tensor.matmul(
                        out=pp,
                        lhsT=bp_row[0:1, 128 * mt : 128 * (mt + 1)],
                        rhs=ones_row[0:1, CS[n]],
                        start=False,
                        stop=True,
                    )
                    nc.vector.tensor_add(ot[:, CS[n]], pp, xs[mt][:, CS[n]])
                nc.sync.dma_start(
                    out=out_d.ap()[128 * mt : 128 * (mt + 1), :], in_=ot
                )

    nc.compile()
    return nc


def prep_inputs(x, gn_scale, gn_bias, qkv_w, qkv_b, proj_w, proj_b):
    """Host-side rearrangement into the per-core input map (shared across cores
    except x)."""
    x = np.asarray(x, dtype=np.float32)
    qkv_w = np.asarray(qkv_w, dtype=np.float32)
    qkv_b = np.asarray(qkv_b, dtype=np.float32)
    proj_w = np.asarray(proj_w, dtype=np.float32)
    proj_b = np.asarray(proj_b, dtype=np.float32)
    gn_scale = np.asarray(gn_scale, dtype=np.float32)
    gn_bias = np.asarray(gn_bias, dtype=np.float32)

    wq3 = qkv_w.reshape(NH, 3 * CH, C)  # per head: [q(64); k(64); v(64)] rows
    q_rows = wq3[:, 0:CH, :]  # [8, 64, 512]
    k_rows = wq3[:, CH : 2 * CH, :]
    v_rows = wq3[:, 2 * CH : 3 * CH, :]
    b3 = qkv_b.reshape(NH, 3 * CH)
    qb, kb, vb = b3[:, 0:CH], b3[:, CH : 2 * CH], b3[:, 2 * CH : 3 * CH]

    # wqk columns: per pair p: [q_2p | q_2p+1 | k_2p | k_2p+1] (128+128)
    cols = []
    bqk = []
    for p in range(NPAIR):
        cols += [q_rows[2 * p], q_rows[2 * p + 1], k_rows[2 * p], k_rows[2 * p + 1]]
        bqk += [qb[2 * p], qb[2 * p + 1], kb[2 * p], kb[2 * p + 1]]
    wqk = np.concatenate(cols, axis=0).T.copy()  # [512, 1024]
    bqk = np.concatenate(bqk).reshape(8, 128)

    wv = v_rows.reshape(C, C).T.copy()  # [512, 512] (c, c'-head-major)
    bv = vb.reshape(1, C).copy()
    wp = proj_w.T.copy()
    bp = proj_b.reshape(1, C).copy()

    ind = np.zeros((NCT, 128, GROUPS), dtype=np.float32)
    for i in range(NCT):
        for cl in range(128):
            ind[i, cl, 8 * i + cl // GSIZE] = 1.0
    indt = np.ascontiguousarray(ind.transpose(0, 2, 1))

    import ml_dtypes

    bf16 = ml_dtypes.bfloat16
    shared = {
        "wqk": wqk.astype(bf16), "wv": wv.astype(bf16), "wp": wp.astype(bf16),
        "bqk": bqk, "bv": bv.astype(bf16), "bp": bp.astype(bf16),
        "gs": np.ascontiguousarray(gn_scale.reshape(NCT, 128)),
        "gb": np.ascontiguousarray(gn_bias.reshape(NCT, 128)),
        "ind": ind.astype(bf16), "indt": indt,
    }
    in_maps = []
    for b in range(B):
        m = dict(shared)
        m["x"] = np.ascontiguousarray(x[b].reshape(C, T))
        in_maps.append(m)
    return in_maps


_NC_CACHE = {}


def _get_nc():
    if "nc" not in _NC_CACHE:
        _NC_CACHE["nc"] = build_nc()
    return _NC_CACHE["nc"]


def kernel(x, gn_scale, gn_bias, qkv_w, qkv_b, proj_w, proj_b, **run_kwargs):
    nc = _get_nc()
    in_maps = prep_inputs(x, gn_scale, gn_bias, qkv_w, qkv_b, proj_w, proj_b)
    res = run_bass_kernel_spmd(nc, in_maps, core_ids=list(range(B)), **run_kwargs)
    out = np.stack([res.results[b]["out"] for b in range(B)])
    kernel.last_results = res
    return out.reshape(B, C, H, W)



# revision 6
# speedup vs baseline: 1.1743x; 1.1743x over previous
"""Trainium2 Bass kernel for nn_AttentionBlock (GroupNorm -> QKV -> 8-head
attention over T=1024 -> proj -> residual) on x[8, 512, 32, 32] f32.

Sharding: data-parallel over batch: core b handles sample b. No collectives.

v2 design (from NTFF trace analysis of v1 @198.7us):
  - ScalarE exp throughput governs the attention phase. v1 used 128 N=512
    ACTIVATEs (865ns each, ~40% fixed overhead). v2 iterates per (parity,
    st): scores land in one 2-bank [128,1024] PSUM tile -> ONE N=1024 exp
    per iteration (64 total), ping-ponged (bufs=2) so ScalarE never idles.
  - GroupNorm stats via DVE tensor_reduce (sum x) + ScalarE Square with
    accum_out (sum x^2): kills all 16 N=512 indicator stats matmuls;
    group-reduce is 4 tiny N=2 matmuls.
  - No zeroing open/close matmuls: each PSUM region opens with its own
    start=True (per-element has_written semantics, verified on HW).
  - No bias rank-1 matmuls: v-bias folded into proj bias on host
    (bp' = bp + wp @ bv, exact since softmax rows sum to 1); proj bias +
    residual fused into one scalar_tensor_tensor eviction.
  - h accumulates into one 2-bank [128,1024] tile; evicted RAW (copy) so
    the banks free immediately; softmax division (x inv broadcast via
    DRAM roundtrip DMA) applied later as background DVE work.
  - vt/qk(next pair) emission interleaved into the attention loop as
    background matmuls so TensorE stays dense behind the exp pipeline.

All matmul operands bf16 (except tiny f32 GN stat/broadcast matmuls);
PSUM accumulation f32. PSUM budget: score 2x[128,1024] (4 banks) +
hc [128,1024] (2) + sums [128,512] (1) + spare [128,512] (1) = 8 banks.
"""

import numpy as np

import concourse.bacc as bacc
import concourse.bass as bass
import concourse.mybir as mybir
import concourse.tile as tile
from concourse.bass_utils import run_bass_kernel_spmd

F32 = mybir.dt.float32
BF16 = mybir.dt.bfloat16
AF = mybir.ActivationFunctionType
ALU = mybir.AluOpType

B, C, H, W = 8, 512, 32, 32
T = H * W  # 1024
NH = 8  # heads
CH = C // NH  # 64 head channels
GROUPS = 32
GSIZE = C // GROUPS  # 16 channels per group
EPS = 1e-5
NCT = C // 128  # 4 channel tiles
NST = T // 128  # 8 spatial tiles
NPAIR = NH // 2  # 4 head pairs
CS = [slice(0, 512), slice(512, 1024)]


def _bcast_ap(src, n):
    """Partition-broadcast AP: replicate src's single partition n times."""
    ap = [[0, n]] + [list(d) for d in list(src.ap)[1:]]
    return bass.AP(tensor=src.tensor, offset=src.offset, ap=ap)


def build_nc(debug_taps=False):
    nc = bacc.Bacc(
        "TRN2",
        target_bir_lowering=False,
        debug=False,
        enable_asserts=False,
        num_devices=8,
    )

    x_d = nc.dram_tensor("x", [C, T], F32, kind="ExternalInput")
    wqk_d = nc.dram_tensor("wqk", [C, 2 * C], BF16, kind="ExternalInput")
    wv_d = nc.dram_tensor("wv", [C, C], BF16, kind="ExternalInput")
    wp_d = nc.dram_tensor("wp", [C, C], BF16, kind="ExternalInput")
    bqk_d = nc.dram_tensor("bqk", [8, 128], F32, kind="ExternalInput")
    bp2_d = nc.dram_tensor("bp2", [NCT, 128], F32, kind="ExternalInput")
    gs_d = nc.dram_tensor("gs", [NCT, 128], F32, kind="ExternalInput")
    gb_d = nc.dram_tensor("gb", [NCT, 128], F32, kind="ExternalInput")
    ind_d = nc.dram_tensor("ind", [NCT, 128, GROUPS], F32, kind="ExternalInput")
    indt_d = nc.dram_tensor("indt", [NCT, GROUPS, 128], F32, kind="ExternalInput")
    out_d = nc.dram_tensor("out", [C, T], F32, kind="ExternalOutput")
    dbg = {}
    if debug_taps:
        dbg["xn"] = nc.dram_tensor("dbg_xn", [NCT, 128, T], BF16, kind="ExternalOutput")
        dbg["qk"] = nc.dram_tensor("dbg_qk", [8, 128, T], BF16, kind="ExternalOutput")
        dbg["vt"] = nc.dram_tensor("dbg_vt", [NST, 128, C], BF16, kind="ExternalOutput")
        dbg["es"] = nc.dram_tensor("dbg_es", [2, 128, T], BF16, kind="ExternalOutput")
        dbg["sums"] = nc.dram_tensor("dbg_sums", [128, 512], F32, kind="ExternalOutput")
        dbg["ha"] = nc.dram_tensor("dbg_ha", [NCT, 128, T], BF16, kind="ExternalOutput")

    with tile.TileContext(nc) as tc:
        with (
            tc.tile_pool(name="sb", bufs=1) as sb,
            tc.tile_pool(name="ps", bufs=1, space="PSUM") as ps,
        ):
            # ---- input loads: GN-critical first, wp last --------------------
            xs, xns, wqks, wvs, wps = [], [], [], [], []
            inds, indts, gss, gbs, bp2s = [], [], [], [], []
            for i in range(NCT):
                xt = sb.tile([128, T], F32, tag="x", bufs=NCT, name=f"x{i}")
                nc.sync.dma_start(out=xt, in_=x_d.ap()[128 * i : 128 * (i + 1), :])
                xs.append(xt)
                it = sb.tile([128, GROUPS], F32, tag="ind", bufs=NCT, name=f"ind{i}")
                nc.sync.dma_start(out=it, in_=ind_d.ap()[i, :, :])
                inds.append(it)
                itt = sb.tile([GROUPS, 128], F32, tag="indt", bufs=NCT, name=f"indt{i}")
                nc.sync.dma_start(out=itt, in_=indt_d.ap()[i, :, :])
                indts.append(itt)
                gst = sb.tile([128, 1], F32, tag="gs", bufs=NCT, name=f"gs{i}")
                nc.sync.dma_start(out=gst, in_=gs_d.ap()[i, :].unsqueeze(1))
                gss.append(gst)
                gbt = sb.tile([128, 1], F32, tag="gb", bufs=NCT, name=f"gb{i}")
                nc.sync.dma_start(out=gbt, in_=gb_d.ap()[i, :].unsqueeze(1))
                gbs.append(gbt)
            for i in range(NCT):
                wq = sb.tile([128, 2 * C], BF16, tag="wqk", bufs=NCT, name=f"wqk{i}")
                nc.sync.dma_start(out=wq, in_=wqk_d.ap()[128 * i : 128 * (i + 1), :])
                wqks.append(wq)
            for i in range(NCT):
                wv = sb.tile([128, C], BF16, tag="wv", bufs=NCT, name=f"wv{i}")
                nc.sync.dma_start(out=wv, in_=wv_d.ap()[128 * i : 128 * (i + 1), :])
                wvs.append(wv)
            bqks = []
            for mt in range(8):
                bq = sb.tile([128, 1], F32, tag="bqk", bufs=8, name=f"bqk{mt}")
                nc.sync.dma_start(out=bq, in_=bqk_d.ap()[mt, :].unsqueeze(1))
                bqks.append(bq)
            for i in range(NCT):
                wp = sb.tile([128, C], BF16, tag="wp", bufs=NCT, name=f"wp{i}")
                nc.sync.dma_start(out=wp, in_=wp_d.ap()[128 * i : 128 * (i + 1), :])
                wps.append(wp)
                bp2t = sb.tile([128, 1], F32, tag="bp2", bufs=NCT, name=f"bp2_{i}")
                nc.sync.dma_start(out=bp2t, in_=bp2_d.ap()[i, :].unsqueeze(1))
                bp2s.append(bp2t)

            ones_col = sb.tile([128, 1], BF16, tag="ones_c", name="ones_col")
            nc.vector.memset(ones_col, 1.0)

            # ---- GroupNorm statistics ---------------------------------------
            # per channel: sum_t x (DVE reduce) and sum_t x^2 (ScalarE Square
            # with accum_out); group-reduce both via one tiny N=2 matmul/tile.
            gsum = ps.tile([GROUPS, 2], F32, tag="spare", name="gsum")
            sx12s = []
            for i in range(NCT):
                sx12 = sb.tile([128, 2], F32, tag="sx12", bufs=NCT, name=f"sx12_{i}")
                nc.vector.tensor_reduce(
                    out=sx12[:, 0:1], in_=xs[i], axis=mybir.AxisListType.X, op=ALU.add
                )
                sqscr = sb.tile([128, T], BF16, tag="sqscr", bufs=2, name=f"sqscr{i}")
                nc.scalar.activation(
                    out=sqscr, in_=xs[i], func=AF.Square, accum_out=sx12[:, 1:2]
                )
                sx12s.append(sx12)
            for i in range(NCT):
                nc.tensor.matmul(
                    out=gsum, lhsT=inds[i], rhs=sx12s[i],
                    start=(i == 0), stop=(i == NCT - 1),
                )

            inv_n = 1.0 / (GSIZE * T)
            mr32 = sb.tile([GROUPS, 2], F32, tag="gnsm", bufs=8, name="mr32")
            nc.vector.tensor_scalar_mul(out=mr32, in0=gsum, scalar1=inv_n)
            msq = sb.tile([GROUPS, 1], F32, tag="gnsm", bufs=8, name="msq")
            nc.vector.tensor_mul(msq, mr32[:, 0:1], mr32[:, 0:1])
            var = sb.tile([GROUPS, 1], F32, tag="gnsm", bufs=8, name="var")
            nc.vector.tensor_sub(var, mr32[:, 1:2], msq)
            eps_t = sb.tile([GROUPS, 1], F32, tag="gnsm", bufs=8, name="eps_t")
            nc.vector.memset(eps_t, EPS)
            lnv = sb.tile([GROUPS, 1], F32, tag="gnsm", bufs=8, name="lnv")
            nc.scalar.activation(out=lnv, in_=var, func=AF.Ln, bias=eps_t, scale=1.0)
            nc.scalar.activation(out=mr32[:, 1:2], in_=lnv, func=AF.Exp, scale=-0.5)

            for i in range(NCT):
                mrb = ps.tile([128, 2], F32, tag="spare", name=f"mrb{i}")
                nc.tensor.matmul(out=mrb, lhsT=indts[i], rhs=mr32, start=True, stop=True)
                a_t = sb.tile([128, 1], F32, tag="gA", bufs=NCT, name=f"gA{i}")
                nc.vector.tensor_mul(a_t, mrb[:, 1:2], gss[i])
                tmp = sb.tile([128, 1], F32, tag="gT", bufs=2, name=f"gT{i}")
                nc.vector.tensor_mul(tmp, mrb[:, 0:1], a_t)
                b_t = sb.tile([128, 1], F32, tag="gB", bufs=NCT, name=f"gB{i}")
                nc.vector.tensor_sub(b_t, gbs[i], tmp)
                xn = sb.tile([128, T], BF16, tag="xn", bufs=NCT, name=f"xn{i}")
                nc.vector.tensor_scalar(
                    out=xn, in0=xs[i], scalar1=a_t, scalar2=b_t,
                    op0=ALU.mult, op1=ALU.add,
                )
                xns.append(xn)
                if debug_taps:
                    nc.sync.dma_start(out=dbg["xn"].ap()[i], in_=xn)

            # ---- QKV / V emission helpers -----------------------------------
            qks = [None] * 8
            vts = [None] * NST

            def emit_qk_half(mt, n, tag="spare"):
                """One t-chunk of q/k m-tile mt -> qks[mt][:, CS[n]]."""
                if qks[mt] is None:
                    qks[mt] = sb.tile([128, T], BF16, tag="qk", bufs=8, name=f"qk{mt}")
                qp = ps.tile(
                    [128, 512], F32, tag=tag,
                    bufs=(2 if tag == "sc" else None), name=f"qp{mt}_{n}",
                )
                for i in range(NCT):
                    nc.tensor.matmul(
                        out=qp,
                        lhsT=wqks[i][:, 128 * mt : 128 * (mt + 1)],
                        rhs=xns[i][:, CS[n]],
                        start=(i == 0),
                        stop=(i == NCT - 1),
                    )
                nc.vector.tensor_scalar_add(
                    out=qks[mt][:, CS[n]], in0=qp, scalar1=bqks[mt]
                )
                if debug_taps and n == 1:
                    nc.sync.dma_start(out=dbg["qk"].ap()[mt], in_=qks[mt])

            def emit_vt(st, tag="spare"):
                vp = ps.tile(
                    [128, 512], F32, tag=tag,
                    bufs=(2 if tag == "sc" else None), name=f"vp{st}",
                )
                for i in range(NCT):
                    nc.tensor.matmul(
                        out=vp,
                        lhsT=xns[i][:, 128 * st : 128 * (st + 1)],
                        rhs=wvs[i],
                        start=(i == 0),
                        stop=(i == NCT - 1),
                    )
                vt = sb.tile([128, C], BF16, tag="vt", bufs=NST, name=f"vt{st}")
                nc.vector.tensor_copy(vt, vp)
                vts[st] = vt
                if debug_taps:
                    nc.sync.dma_start(out=dbg["vt"].ap()[st], in_=vt)

            # prologue: q/k for pair 0 and vt(0) through the (still idle)
            # score-pool banks for ping-pong; rest is background work.
            for mt in (0, 1):
                for n in range(2):
                    emit_qk_half(mt, n, tag="sc")
            emit_vt(0, tag="sc")

            bg = []  # background thunks, each ~1 matmul-group
            for st in range(1, NST):
                bg.append(lambda st=st: emit_vt(st))
            for mt in (2, 3):
                for n in range(2):
                    bg.append(lambda mt=mt, n=n: emit_qk_half(mt, n))

            # ---- attention ---------------------------------------------------
            has = []
            ha_muls = []  # deferred inv-application thunks
            for p in range(NPAIR):
                qtile, ktile = qks[2 * p], qks[2 * p + 1]
                hc = ps.tile([128, T], F32, tag="hc", name=f"hc{p}")
                sums = ps.tile([128, 512], F32, tag="sums", name=f"sums{p}")

                its = [(par, st) for par in range(2) for st in range(NST)]
                scs, es = {}, {}

                def sc_mms(k, p=p, qtile=qtile, ktile=ktile, scs=scs):
                    par, st = its[k]
                    pr = slice(64 * par, 64 * (par + 1))
                    ss = slice(128 * st, 128 * (st + 1))
                    sc = ps.tile([128, T], F32, tag="sc", bufs=2, name=f"sc{p}_{k}")
                    for n in range(2):
                        nc.tensor.matmul(
                            out=sc[:, CS[n]], lhsT=ktile[pr, ss],
                            rhs=qtile[pr, CS[n]], start=True, stop=True,
                        )
                    scs[k] = sc

                def exp_act(k, p=p, scs=scs, es=es):
                    e = sb.tile([128, T], BF16, tag="es", bufs=3, name=f"e{p}_{k}")
                    nc.scalar.activation(out=e, in_=scs[k], func=AF.Exp, scale=0.125)
                    es[k] = e
                    if debug_taps and p == 0 and k in (0, 8):
                        nc.sync.dma_start(out=dbg["es"].ap()[k // 8], in_=e)

                def h_sums_mms(k, p=p, hc=hc, sums=sums, es=es):
                    par, st = its[k]
                    e = es[k]
                    first, last = st == 0, st == NST - 1
                    vsl = slice(128 * p + 64 * par, 128 * p + 64 * (par + 1))
                    for n in range(2):
                        nc.tensor.matmul(
                            out=hc[64 * par : 64 * (par + 1), CS[n]],
                            lhsT=vts[st][:, vsl], rhs=e[:, CS[n]],
                            start=first, stop=last, skip_group_check=True,
                        )
                    for n in range(2):
                        r = 64 * par + 32 * n
                        nc.tensor.matmul(
                            out=sums[r : r + 1, :], lhsT=ones_col, rhs=e[:, CS[n]],
                            start=first, stop=last, skip_group_check=True,
                            tile_position=(0, r),
                        )

                # software pipeline: exp(k) overlaps PE on h(k-1)+sc(k+2)+bg
                sc_mms(0)
                sc_mms(1)
                exp_act(0)
                for k in range(16):
                    if k + 2 < 16:
                        sc_mms(k + 2)
                    if k + 1 < 16:
                        exp_act(k + 1)
                    h_sums_mms(k)
                    if bg:
                        bg.pop(0)()

                # pair tail: reciprocal of sums rows, DRAM-roundtrip partition
                # broadcast, raw h eviction (frees hc banks immediately);
                # the inv multiply is deferred background DVE work.
                invs = sb.tile([128, 512], F32, tag="invs", bufs=2, name=f"invs{p}")
                nc.vector.reciprocal_approx_fast(out=invs, in_=sums)
                if debug_taps and p == 0:
                    nc.sync.dma_start(out=dbg["sums"].ap(), in_=invs)
                inv = sb.tile([128, T], F32, tag="inv", bufs=2, name=f"inv{p}")
                invd = nc.dram_tensor(f"invd{p}", [4, 512], F32)
                for j, row in enumerate((0, 32, 64, 96)):
                    nc.sync.dma_start(
                        out=invd.ap()[j : j + 1, :], in_=invs[row : row + 1, :]
                    )
                for j, (dp, n) in enumerate(((0, 0), (0, 1), (64, 0), (64, 1))):
                    nc.sync.dma_start(
                        out=inv[dp : dp + 64, CS[n]],
                        in_=_bcast_ap(invd.ap()[j : j + 1, :], 64),
                    )
                har = sb.tile([128, T], F32, tag="har", bufs=2, name=f"har{p}")
                nc.vector.tensor_copy(har, hc)
                ha = sb.tile([128, T], BF16, tag="ha", bufs=NPAIR, name=f"ha{p}")

                def ha_mul(ha=ha, har=har, inv=inv, p=p):
                    nc.vector.tensor_mul(ha, har, inv)
                    if debug_taps:
                        nc.sync.dma_start(out=dbg["ha"].ap()[p], in_=ha)

                ha_muls.append(ha_mul)
                has.append(ha)
                if p < NPAIR - 1:
                    bg = []
                    if p + 2 <= NPAIR - 1:
                        for mt in (2 * p + 4, 2 * p + 5):
                            for n in range(2):
                                bg.append(
                                    lambda mt=mt, n=n: emit_qk_half(mt, n)
                                )
                    # apply previous pair's inv while this pair streams
                    bg.append(ha_muls[p])

            ha_muls[NPAIR - 1]()

            # ---- proj + residual --------------------------------------------
            for mt in range(NCT):
                pp = ps.tile([128, T], F32, tag="sc", bufs=2, name=f"pp{mt}")
                for i in range(NCT):
                    for n in range(2):
                        nc.tensor.matmul(
                            out=pp[:, CS[n]],
                            lhsT=wps[i][:, 128 * mt : 128 * (mt + 1)],
                            rhs=has[i][:, CS[n]],
                            start=(i == 0),
                            stop=(i == NCT - 1),
                        )
                ot = sb.tile([128, T], F32, tag="ot", bufs=2, name=f"ot{mt}")
                nc.vector.scalar_tensor_tensor(
                    out=ot, in0=pp, scalar=bp2s[mt], in1=xs[mt],
                    op0=ALU.add, op1=ALU.add,
                )
                nc.sync.dma_start(
                    out=out_d.ap()[128 * mt : 128 * (mt + 1), :], in_=ot
                )

    nc.compile()
    return nc


def prep_inputs(x, gn_scale, gn_bias, qkv_w, qkv_b, proj_w, proj_b):
    """Host-side rearrangement into the per-core input map (shared across cores
    except x)."""
    x = np.asarray(x, dtype=np.float32)
    qkv_w = np.asarray(qkv_w, dtype=np.float32)
    qkv_b = np.asarray(qkv_b, dtype=np.float32)
    proj_w = np.asarray(proj_w, dtype=np.float32)
    proj_b = np.asarray(proj_b, dtype=np.float32)
    gn_scale = np.asarray(gn_scale, dtype=np.float32)
    gn_bias = np.asarray(gn_bias, dtype=np.float32)

    wq3 = qkv_w.reshape(NH, 3 * CH, C)  # per head: [q(64); k(64); v(64)] rows
    q_rows = wq3[:, 0:CH, :]  # [8, 64, 512]
    k_rows = wq3[:, CH : 2 * CH, :]
    v_rows = wq3[:, 2 * CH : 3 * CH, :]
    b3 = qkv_b.reshape(NH, 3 * CH)
    qb, kb, vb = b3[:, 0:CH], b3[:, CH : 2 * CH], b3[:, 2 * CH : 3 * CH]

    # wqk columns: per pair p: [q_2p | q_2p+1 | k_2p | k_2p+1] (128+128)
    cols = []
    bqk = []
    for p in range(NPAIR):
        cols += [q_rows[2 * p], q_rows[2 * p + 1], k_rows[2 * p], k_rows[2 * p + 1]]
        bqk += [qb[2 * p], qb[2 * p + 1], kb[2 * p], kb[2 * p + 1]]
    wqk = np.concatenate(cols, axis=0).T.copy()  # [512, 1024]
    bqk = np.concatenate(bqk).reshape(8, 128)

    wv = v_rows.reshape(C, C).T.copy()  # [512, 512] (c, c'-head-major)
    wp = proj_w.T.copy()
    # v-bias folded into proj bias: softmax rows sum to 1, so
    # h_withbias = h + bv  =>  proj(h) + proj_w @ bv + proj_b.
    vb_hm = vb.reshape(C)  # head-major v bias, matches proj_w columns
    bp2 = (proj_b + proj_w @ vb_hm).reshape(NCT, 128)

    ind = np.zeros((NCT, 128, GROUPS), dtype=np.float32)
    for i in range(NCT):
        for cl in range(128):
            ind[i, cl, 8 * i + cl // GSIZE] = 1.0
    indt = np.ascontiguousarray(ind.transpose(0, 2, 1))

    import ml_dtypes

    bf16 = ml_dtypes.bfloat16
    shared = {
        "wqk": wqk.astype(bf16), "wv": wv.astype(bf16), "wp": wp.astype(bf16),
        "bqk": bqk, "bp2": np.ascontiguousarray(bp2),
        "gs": np.ascontiguousarray(gn_scale.reshape(NCT, 128)),
        "gb": np.ascontiguousarray(gn_bias.reshape(NCT, 128)),
        "ind": ind, "indt": indt,
    }
    in_maps = []
    for b in range(B):
        m = dict(shared)
        m["x"] = np.ascontiguousarray(x[b].reshape(C, T))
        in_maps.append(m)
    return in_maps


_NC_CACHE = {}


def _get_nc():
    if "nc" not in _NC_CACHE:
        _NC_CACHE["nc"] = build_nc()
    return _NC_CACHE["nc"]


def kernel(x, gn_scale, gn_bias, qkv_w, qkv_b, proj_w, proj_b, **run_kwargs):
    nc = _get_nc()
    in_maps = prep_inputs(x, gn_scale, gn_bias, qkv_w, qkv_b, proj_w, proj_b)
    res = run_bass_kernel_spmd(nc, in_maps, core_ids=list(range(B)), **run_kwargs)
    out = np.stack([res.results[b]["out"] for b in range(B)])
    kernel.last_results = res
    return out.reshape(B, C, H, W)


# revision 13
# speedup vs baseline: 1.1913x; 1.0145x over previous
"""Trainium2 Bass kernel for nn_AttentionBlock (GroupNorm -> QKV -> 8-head
attention over T=1024 -> proj -> residual) on x[8, 512, 32, 32] f32.

Sharding: data-parallel over batch: core b handles sample b. No collectives.

v2 design (from NTFF trace analysis of v1 @198.7us):
  - ScalarE exp throughput governs the attention phase. v1 used 128 N=512
    ACTIVATEs (865ns each, ~40% fixed overhead). v2 iterates per (parity,
    st): scores land in one 2-bank [128,1024] PSUM tile -> ONE N=1024 exp
    per iteration (64 total), ping-ponged (bufs=2) so ScalarE never idles.
  - GroupNorm stats via DVE tensor_reduce (sum x) + ScalarE Square with
    accum_out (sum x^2): kills all 16 N=512 indicator stats matmuls;
    group-reduce is 4 tiny N=2 matmuls.
  - No zeroing open/close matmuls: each PSUM region opens with its own
    start=True (per-element has_written semantics, verified on HW).
  - No bias rank-1 matmuls: v-bias folded into proj bias on host
    (bp' = bp + wp @ bv, exact since softmax rows sum to 1); proj bias +
    residual fused into one scalar_tensor_tensor eviction.
  - h accumulates into one 2-bank [128,1024] tile; evicted RAW (copy) so
    the banks free immediately; softmax division (x inv broadcast via
    DRAM roundtrip DMA) applied later as background DVE work.
  - vt/qk(next pair) emission interleaved into the attention loop as
    background matmuls so TensorE stays dense behind the exp pipeline.

All matmul operands bf16 (except tiny f32 GN stat/broadcast matmuls);
PSUM accumulation f32. PSUM budget: score 2x[128,1024] (4 banks) +
hc [128,1024] (2) + sums [128,512] (1) + spare [128,512] (1) = 8 banks.
"""

import numpy as np

import concourse.bacc as bacc
import concourse.bass as bass
import concourse.mybir as mybir
import concourse.tile as tile
from concourse.bass_utils import run_bass_kernel_spmd

F32 = mybir.dt.float32
BF16 = mybir.dt.bfloat16
AF = mybir.ActivationFunctionType
ALU = mybir.AluOpType

B, C, H, W = 8, 512, 32, 32
T = H * W  # 1024
NH = 8  # heads
CH = C // NH  # 64 head channels
GROUPS = 32
GSIZE = C // GROUPS  # 16 channels per group
EPS = 1e-5
NCT = C // 128  # 4 channel tiles
NST = T // 128  # 8 spatial tiles
NPAIR = NH // 2  # 4 head pairs
CS = [slice(0, 512), slice(512, 1024)]


def _bcast_ap(src, n):
    """Partition-broadcast AP: replicate src's single partition n times."""
    ap = [[0, n]] + [list(d) for d in list(src.ap)[1:]]
    return bass.AP(tensor=src.tensor, offset=src.offset, ap=ap)


def build_nc(debug_taps=False):
    nc = bacc.Bacc(
        "TRN2",
        target_bir_lowering=False,
        debug=False,
        enable_asserts=False,
        num_devices=8,
    )

    x_d = nc.dram_tensor("x", [C, T], F32, kind="ExternalInput")
    wqk_d = nc.dram_tensor("wqk", [C, 2 * C], BF16, kind="ExternalInput")
    wv_d = nc.dram_tensor("wv", [C, C], BF16, kind="ExternalInput")
    wp_d = nc.dram_tensor("wp", [C, C], BF16, kind="ExternalInput")
    bqk_d = nc.dram_tensor("bqk", [8, 128], F32, kind="ExternalInput")
    bp2_d = nc.dram_tensor("bp2", [NCT, 128], F32, kind="ExternalInput")
    gs_d = nc.dram_tensor("gs", [NCT, 128], F32, kind="ExternalInput")
    gb_d = nc.dram_tensor("gb", [NCT, 128], F32, kind="ExternalInput")
    ind_d = nc.dram_tensor("ind", [NCT, 128, GROUPS], F32, kind="ExternalInput")
    indt_d = nc.dram_tensor("indt", [NCT, GROUPS, 128], F32, kind="ExternalInput")
    out_d = nc.dram_tensor("out", [C, T], F32, kind="ExternalOutput")
    dbg = {}
    if debug_taps:
        dbg["xn"] = nc.dram_tensor("dbg_xn", [NCT, 128, T], BF16, kind="ExternalOutput")
        dbg["qk"] = nc.dram_tensor("dbg_qk", [8, 128, T], BF16, kind="ExternalOutput")
        dbg["vt"] = nc.dram_tensor("dbg_vt", [NST, 128, C], BF16, kind="ExternalOutput")
        dbg["es"] = nc.dram_tensor("dbg_es", [2, 128, T], BF16, kind="ExternalOutput")
        dbg["sums"] = nc.dram_tensor("dbg_sums", [128, 512], F32, kind="ExternalOutput")
        dbg["ha"] = nc.dram_tensor("dbg_ha", [NCT, 128, T], BF16, kind="ExternalOutput")

    with tile.TileContext(nc) as tc:
        with (
            tc.tile_pool(name="sb", bufs=1) as sb,
            tc.tile_pool(name="ps", bufs=1, space="PSUM") as ps,
        ):
            # ---- input loads: x -> ind -> wqk -> rest (arrival-order = need
            # order: stats need x+ind, qk needs wqk, vt needs wv) -------------
            xs, xns, wqks, wvs, wps = [], [], [], [], []
            inds, indts, gss, gbs, bp2s = [], [], [], [], []
            for i in range(NCT):
                xt = sb.tile([128, T], F32, tag="x", bufs=NCT, name=f"x{i}")
                nc.sync.dma_start(out=xt, in_=x_d.ap()[128 * i : 128 * (i + 1), :])
                xs.append(xt)
            for i in range(NCT):
                it = sb.tile([128, GROUPS], F32, tag="ind", bufs=NCT, name=f"ind{i}")
                nc.sync.dma_start(out=it, in_=ind_d.ap()[i, :, :])
                inds.append(it)
            for i in range(NCT):
                wq = sb.tile([128, 2 * C], BF16, tag="wqk", bufs=NCT, name=f"wqk{i}")
                nc.sync.dma_start(out=wq, in_=wqk_d.ap()[128 * i : 128 * (i + 1), :])
                wqks.append(wq)
            for i in range(NCT):
                itt = sb.tile([GROUPS, 128], F32, tag="indt", bufs=NCT, name=f"indt{i}")
                nc.sync.dma_start(out=itt, in_=indt_d.ap()[i, :, :])
                indts.append(itt)
                gst = sb.tile([128, 1], F32, tag="gs", bufs=NCT, name=f"gs{i}")
                nc.sync.dma_start(out=gst, in_=gs_d.ap()[i, :].unsqueeze(1))
                gss.append(gst)
                gbt = sb.tile([128, 1], F32, tag="gb", bufs=NCT, name=f"gb{i}")
                nc.sync.dma_start(out=gbt, in_=gb_d.ap()[i, :].unsqueeze(1))
                gbs.append(gbt)
            for i in range(NCT):
                wv = sb.tile([128, C], BF16, tag="wv", bufs=NCT, name=f"wv{i}")
                nc.sync.dma_start(out=wv, in_=wv_d.ap()[128 * i : 128 * (i + 1), :])
                wvs.append(wv)
            bqks = []
            for mt in range(8):
                bq = sb.tile([128, 1], F32, tag="bqk", bufs=8, name=f"bqk{mt}")
                nc.sync.dma_start(out=bq, in_=bqk_d.ap()[mt, :].unsqueeze(1))
                bqks.append(bq)
            for i in range(NCT):
                wp = sb.tile([128, C], BF16, tag="wp", bufs=NCT, name=f"wp{i}")
                nc.sync.dma_start(out=wp, in_=wp_d.ap()[128 * i : 128 * (i + 1), :])
                wps.append(wp)
                bp2t = sb.tile([128, 1], F32, tag="bp2", bufs=NCT, name=f"bp2_{i}")
                nc.sync.dma_start(out=bp2t, in_=bp2_d.ap()[i, :].unsqueeze(1))
                bp2s.append(bp2t)

            ones_col = sb.tile([128, 1], BF16, tag="ones_c", name="ones_col")
            nc.vector.memset(ones_col, 1.0)
            ones_row = sb.tile([1, 512], BF16, tag="ones_r", name="ones_row")
            nc.vector.memset(ones_row, 1.0)

            # ---- HAM warmup: dense rank-1 matmul burst while DMAs land ------
            # PE_HAM unthrottles (1.2 -> 2.4 GHz) only after ~3.4us of
            # sustained PE activity; burn ~5.5us on a never-read accumulator
            # so the real head matmuls run at full clock.
            wu = ps.tile([1, 512], F32, tag="spare", name="wu")
            for j in range(26):
                nc.tensor.matmul(
                    out=wu, lhsT=ones_row[0:1, 0:1], rhs=ones_row,
                    start=(j == 0), stop=(j == 25),
                )

            # ---- GroupNorm statistics ---------------------------------------
            # per channel: sum_t x (DVE reduce) and sum_t x^2 (ScalarE Square
            # with accum_out); group-reduce both via one tiny N=2 matmul/tile.
            gsum = ps.tile([GROUPS, 2], F32, tag="spare", name="gsum")
            sx12s = []
            for i in range(NCT):
                sx12 = sb.tile([128, 2], F32, tag="sx12", bufs=NCT, name=f"sx12_{i}")
                nc.vector.tensor_reduce(
                    out=sx12[:, 0:1], in_=xs[i], axis=mybir.AxisListType.X, op=ALU.add
                )
                sqscr = sb.tile([128, T], BF16, tag="sqscr", bufs=2, name=f"sqscr{i}")
                nc.scalar.activation(
                    out=sqscr, in_=xs[i], func=AF.Square, accum_out=sx12[:, 1:2]
                )
                sx12s.append(sx12)
            for i in range(NCT):
                nc.tensor.matmul(
                    out=gsum, lhsT=inds[i], rhs=sx12s[i],
                    start=(i == 0), stop=(i == NCT - 1),
                )

            inv_n = 1.0 / (GSIZE * T)
            mr32 = sb.tile([GROUPS, 2], F32, tag="gnsm", bufs=8, name="mr32")
            nc.vector.tensor_scalar_mul(out=mr32, in0=gsum, scalar1=inv_n)
            msq = sb.tile([GROUPS, 1], F32, tag="gnsm", bufs=8, name="msq")
            nc.vector.tensor_mul(msq, mr32[:, 0:1], mr32[:, 0:1])
            var = sb.tile([GROUPS, 1], F32, tag="gnsm", bufs=8, name="var")
            nc.vector.tensor_sub(var, mr32[:, 1:2], msq)
            eps_t = sb.tile([GROUPS, 1], F32, tag="gnsm", bufs=8, name="eps_t")
            nc.vector.memset(eps_t, EPS)
            lnv = sb.tile([GROUPS, 1], F32, tag="gnsm", bufs=8, name="lnv")
            nc.scalar.activation(out=lnv, in_=var, func=AF.Ln, bias=eps_t, scale=1.0)
            nc.scalar.activation(out=mr32[:, 1:2], in_=lnv, func=AF.Exp, scale=-0.5)

            for i in range(NCT):
                mrb = ps.tile([128, 2], F32, tag="spare", name=f"mrb{i}")
                nc.tensor.matmul(out=mrb, lhsT=indts[i], rhs=mr32, start=True, stop=True)
                a_t = sb.tile([128, 1], F32, tag="gA", bufs=NCT, name=f"gA{i}")
                nc.vector.tensor_mul(a_t, mrb[:, 1:2], gss[i])
                tmp = sb.tile([128, 1], F32, tag="gT", bufs=2, name=f"gT{i}")
                nc.vector.tensor_mul(tmp, mrb[:, 0:1], a_t)
                b_t = sb.tile([128, 1], F32, tag="gB", bufs=NCT, name=f"gB{i}")
                nc.vector.tensor_sub(b_t, gbs[i], tmp)
                xn = sb.tile([128, T], BF16, tag="xn", bufs=NCT, name=f"xn{i}")
                nc.vector.tensor_scalar(
                    out=xn, in0=xs[i], scalar1=a_t, scalar2=b_t,
                    op0=ALU.mult, op1=ALU.add,
                )
                xns.append(xn)
                if debug_taps:
                    nc.sync.dma_start(out=dbg["xn"].ap()[i], in_=xn)

            # ---- QKV / V emission helpers -----------------------------------
            qks = [None] * 8
            vts = [None] * NST

            def emit_qk_half(mt, n, tag="spare"):
                """One t-chunk of q/k m-tile mt -> qks[mt][:, CS[n]]."""
                if qks[mt] is None:
                    qks[mt] = sb.tile([128, T], BF16, tag="qk", bufs=8, name=f"qk{mt}")
                qp = ps.tile(
                    [128, 512], F32, tag=tag,
                    bufs=(2 if tag == "sc" else None), name=f"qp{mt}_{n}",
                )
                for i in range(NCT):
                    nc.tensor.matmul(
                        out=qp,
                        lhsT=wqks[i][:, 128 * mt : 128 * (mt + 1)],
                        rhs=xns[i][:, CS[n]],
                        start=(i == 0),
                        stop=(i == NCT - 1),
                    )
                nc.vector.tensor_scalar_add(
                    out=qks[mt][:, CS[n]], in0=qp, scalar1=bqks[mt]
                )
                if debug_taps and n == 1:
                    nc.sync.dma_start(out=dbg["qk"].ap()[mt], in_=qks[mt])

            def emit_vt_part(st, ilo, ihi, tag="spare"):
                vp = ps.tile(
                    [128, 512], F32, tag=tag,
                    bufs=(2 if tag == "sc" else None), name=f"vp{st}",
                ) if ilo == 0 else emit_vt_part.vp
                emit_vt_part.vp = vp
                for i in range(ilo, ihi):
                    nc.tensor.matmul(
                        out=vp,
                        lhsT=xns[i][:, 128 * st : 128 * (st + 1)],
                        rhs=wvs[i],
                        start=(i == 0),
                        stop=(i == NCT - 1),
                    )
                if ihi == NCT:
                    vt = sb.tile([128, C], BF16, tag="vt", bufs=NST, name=f"vt{st}")
                    nc.vector.tensor_copy(vt, vp)
                    vts[st] = vt
                    if debug_taps:
                        nc.sync.dma_start(out=dbg["vt"].ap()[st], in_=vt)

            def emit_vt(st, tag="spare"):
                emit_vt_part(st, 0, NCT, tag=tag)

            def emit_qk_part(mt, n, ilo, ihi, tag="spare"):
                """K-subrange [ilo,ihi) of one qk half; evict when ihi==NCT."""
                qp = ps.tile(
                    [128, 512], F32, tag=tag,
                    bufs=(2 if tag == "sc" else None), name=f"qp{mt}_{n}",
                ) if ilo == 0 else emit_qk_part.qp
                emit_qk_part.qp = qp
                for i in range(ilo, ihi):
                    nc.tensor.matmul(
                        out=qp,
                        lhsT=wqks[i][:, 128 * mt : 128 * (mt + 1)],
                        rhs=xns[i][:, CS[n]],
                        start=(i == 0),
                        stop=(i == NCT - 1),
                    )
                if ihi == NCT:
                    nc.vector.tensor_scalar_add(
                        out=qks[mt][:, CS[n]], in0=qp, scalar1=bqks[mt]
                    )

            # prologue: q/k for pair 0 and vt 0-3 through the (still idle)
            # score-pool banks; vt matmuls fill PE gaps left by qk evictions.
            emit_qk_half(0, 0, tag="sc")
            emit_vt(1, tag="sc")
            emit_qk_half(0, 1, tag="sc")
            emit_vt(2, tag="sc")
            emit_qk_half(1, 0, tag="sc")
            emit_vt(3, tag="sc")
            emit_qk_half(1, 1, tag="sc")
            emit_vt(0, tag="sc")

            # background thunks, one popped per attention slot
            order = []
            for st in range(4, NST):
                order.append(lambda st=st: emit_vt(st))
            for mt in range(2, 8):
                qks[mt] = sb.tile([128, T], BF16, tag="qk", bufs=8, name=f"qk{mt}")
                for n in range(2):
                    order.append(lambda mt=mt, n=n: emit_qk_part(mt, n, 0, 2))
                    order.append(lambda mt=mt, n=n: emit_qk_part(mt, n, 2, 4))

            # ---- attention: flat 64-slot pipeline across all pairs ----------
            has, hcs, sums_l, pair_objs = [], [], [], []
            scs, es = {}, {}

            def pair_state(p):
                hc = ps.tile([128, T], F32, tag="hc", name=f"hc{p}")
                sums = ps.tile([128, 512], F32, tag="sums", name=f"sums{p}")
                hcs.append(hc)
                sums_l.append(sums)

            def sc_mms(K):
                p, k = K // 16, K % 16
                par, st = k // 8, k % 8
                qtile, ktile = qks[2 * p], qks[2 * p + 1]
                pr = slice(64 * par, 64 * (par + 1))
                ss = slice(128 * st, 128 * (st + 1))
                sc = ps.tile([128, T], F32, tag="sc", bufs=2, name=f"sc{K}")
                for n in range(2):
                    nc.tensor.matmul(
                        out=sc[:, CS[n]], lhsT=ktile[pr, ss],
                        rhs=qtile[pr, CS[n]], start=True, stop=True,
                    )
                scs[K] = sc

            def exp_act(K):
                e = sb.tile([128, T], BF16, tag="es", bufs=3, name=f"e{K}")
                nc.scalar.activation(out=e, in_=scs[K], func=AF.Exp, scale=0.125)
                es[K] = e
                if debug_taps and K in (0, 8):
                    nc.sync.dma_start(out=dbg["es"].ap()[K // 8], in_=e)

            def h_sums_mms(K):
                p, k = K // 16, K % 16
                par, st = k // 8, k % 8
                e = es.pop(K)
                hc, sums = hcs[p], sums_l[p]
                first, last = st == 0, st == NST - 1
                vsl = slice(128 * p + 64 * par, 128 * p + 64 * (par + 1))
                for n in range(2):
                    nc.tensor.matmul(
                        out=hc[64 * par : 64 * (par + 1), CS[n]],
                        lhsT=vts[st][:, vsl], rhs=e[:, CS[n]],
                        start=first, stop=last, skip_group_check=True,
                    )
                for n in range(2):
                    r = 64 * par + 32 * n
                    nc.tensor.matmul(
                        out=sums[r : r + 1, :], lhsT=ones_col, rhs=e[:, CS[n]],
                        start=first, stop=last, skip_group_check=True,
                        tile_position=(0, r),
                    )

            def pair_tail(p):
                """Emit right after h_sums of slot 16p+15 (cross-pair sc/exp
                for p+1 already issued). DRAM-roundtrip partition broadcast
                of 1/sums; raw hc eviction frees the banks; inv multiply is
                deferred to background."""
                invs = sb.tile([128, 512], F32, tag="invs", bufs=2, name=f"invs{p}")
                nc.vector.reciprocal_approx_fast(out=invs, in_=sums_l[p])
                if debug_taps and p == 0:
                    nc.sync.dma_start(out=dbg["sums"].ap(), in_=invs)
                har = sb.tile([128, T], F32, tag="har", bufs=2, name=f"har{p}")
                nc.vector.tensor_copy(har, hcs[p])
                ha = sb.tile([128, T], BF16, tag="ha", bufs=NPAIR, name=f"ha{p}")
                has.append(ha)
                inv = sb.tile([128, T], F32, tag="inv", bufs=2, name=f"inv{p}")
                invd = nc.dram_tensor(f"invd{p}", [4, 512], F32)
                for j, row in enumerate((0, 32, 64, 96)):
                    nc.sync.dma_start(
                        out=invd.ap()[j : j + 1, :], in_=invs[row : row + 1, :]
                    )
                for j, (dp, n) in enumerate(((0, 0), (0, 1), (64, 0), (64, 1))):
                    nc.sync.dma_start(
                        out=inv[dp : dp + 64, CS[n]],
                        in_=_bcast_ap(invd.ap()[j : j + 1, :], 64),
                    )

                def ha_mul(ha=ha, har=har, inv=inv, p=p):
                    nc.vector.tensor_mul(ha, har, inv)
                    if debug_taps:
                        nc.sync.dma_start(out=dbg["ha"].ap()[p], in_=ha)

                if p < NPAIR - 1:
                    order.append(ha_mul)
                else:
                    pair_objs.append(ha_mul)

            # pipeline: sc two ahead, exp one ahead, h/sums at K, 1 bg thunk
            pair_state(0)
            sc_mms(0)
            sc_mms(1)
            exp_act(0)
            for K in range(64):
                if K % 16 == 15 and K // 16 < 3:
                    pair_state(K // 16 + 1)
                if K + 2 < 64:
                    sc_mms(K + 2)
                if K + 1 < 64:
                    exp_act(K + 1)
                h_sums_mms(K)
                if K % 16 == 15:
                    pair_tail(K // 16)
                if order:
                    order.pop(0)()

            while order:
                order.pop(0)()

            # ---- proj + residual (pair-3 K-tiles last: its ha lands late) ---
            pps = {}

            def start_proj(mt, tag):
                pp = ps.tile(
                    [128, T], F32, tag=tag,
                    bufs=(2 if tag == "sc" else None), name=f"pp{mt}",
                )
                pps[mt] = pp
                for i in range(NCT - 1):
                    for n in range(2):
                        nc.tensor.matmul(
                            out=pp[:, CS[n]],
                            lhsT=wps[i][:, 128 * mt : 128 * (mt + 1)],
                            rhs=has[i][:, CS[n]],
                            start=(i == 0),
                            stop=False,
                        )

            def finish_proj(mt):
                pp = pps[mt]
                for n in range(2):
                    nc.tensor.matmul(
                        out=pp[:, CS[n]],
                        lhsT=wps[3][:, 128 * mt : 128 * (mt + 1)],
                        rhs=has[3][:, CS[n]],
                        start=False,
                        stop=True,
                    )
                ot = sb.tile([128, T], F32, tag="ot", bufs=2, name=f"ot{mt}")
                nc.vector.scalar_tensor_tensor(
                    out=ot, in0=pp, scalar=bp2s[mt], in1=xs[mt],
                    op0=ALU.add, op1=ALU.add,
                )
                nc.sync.dma_start(
                    out=out_d.ap()[128 * mt : 128 * (mt + 1), :], in_=ot
                )

            # pairs 0-2 accumulate while pair-3's inv multiply is in flight
            start_proj(0, "sc")
            start_proj(1, "hc")
            pair_objs[0]()  # ha_mul3 on DVE
            finish_proj(0)
            finish_proj(1)
            start_proj(2, "sc")
            finish_proj(2)
            start_proj(3, "sc")
            finish_proj(3)

    nc.compile()
    return nc


def prep_inputs(x, gn_scale, gn_bias, qkv_w, qkv_b, proj_w, proj_b):
    """Host-side rearrangement into the per-core input map (shared across cores
    except x)."""
    x = np.asarray(x, dtype=np.float32)
    qkv_w = np.asarray(qkv_w, dtype=np.float32)
    qkv_b = np.asarray(qkv_b, dtype=np.float32)
    proj_w = np.asarray(proj_w, dtype=np.float32)
    proj_b = np.asarray(proj_b, dtype=np.float32)
    gn_scale = np.asarray(gn_scale, dtype=np.float32)
    gn_bias = np.asarray(gn_bias, dtype=np.float32)

    wq3 = qkv_w.reshape(NH, 3 * CH, C)  # per head: [q(64); k(64); v(64)] rows
    q_rows = wq3[:, 0:CH, :]  # [8, 64, 512]
    k_rows = wq3[:, CH : 2 * CH, :]
    v_rows = wq3[:, 2 * CH : 3 * CH, :]
    b3 = qkv_b.reshape(NH, 3 * CH)
    qb, kb, vb = b3[:, 0:CH], b3[:, CH : 2 * CH], b3[:, 2 * CH : 3 * CH]

    # wqk columns: per pair p: [q_2p | q_2p+1 | k_2p | k_2p+1] (128+128)
    cols = []
    bqk = []
    for p in range(NPAIR):
        cols += [q_rows[2 * p], q_rows[2 * p + 1], k_rows[2 * p], k_rows[2 * p + 1]]
        bqk += [qb[2 * p], qb[2 * p + 1], kb[2 * p], kb[2 * p + 1]]
    wqk = np.concatenate(cols, axis=0).T.copy()  # [512, 1024]
    bqk = np.concatenate(bqk).reshape(8, 128)

    wv = v_rows.reshape(C, C).T.copy()  # [512, 512] (c, c'-head-major)
    wp = proj_w.T.copy()
    # v-bias folded into proj bias: softmax rows sum to 1, so
    # h_withbias = h + bv  =>  proj(h) + proj_w @ bv + proj_b.
    vb_hm = vb.reshape(C)  # head-major v bias, matches proj_w columns
    bp2 = (proj_b + proj_w @ vb_hm).reshape(NCT, 128)

    ind = np.zeros((NCT, 128, GROUPS), dtype=np.float32)
    for i in range(NCT):
        for cl in range(128):
            ind[i, cl, 8 * i + cl // GSIZE] = 1.0
    indt = np.ascontiguousarray(ind.transpose(0, 2, 1))

    import ml_dtypes

    bf16 = ml_dtypes.bfloat16
    shared = {
        "wqk": wqk.astype(bf16), "wv": wv.astype(bf16), "wp": wp.astype(bf16),
        "bqk": bqk, "bp2": np.ascontiguousarray(bp2),
        "gs": np.ascontiguousarray(gn_scale.reshape(NCT, 128)),
        "gb": np.ascontiguousarray(gn_bias.reshape(NCT, 128)),
        "ind": ind, "indt": indt,
    }
    in_maps = []
    for b in range(B):
        m = dict(shared)
        m["x"] = np.ascontiguousarray(x[b].reshape(C, T))
        in_maps.append(m)
    return in_maps


_NC_CACHE = {}


def _get_nc():
    if "nc" not in _NC_CACHE:
        _NC_CACHE["nc"] = build_nc()
    return _NC_CACHE["nc"]


def kernel(x, gn_scale, gn_bias, qkv_w, qkv_b, proj_w, proj_b, **run_kwargs):
    nc = _get_nc()
    in_maps = prep_inputs(x, gn_scale, gn_bias, qkv_w, qkv_b, proj_w, proj_b)
    res = run_bass_kernel_spmd(nc, in_maps, core_ids=list(range(B)), **run_kwargs)
    out = np.stack([res.results[b]["out"] for b in range(B)])
    kernel.last_results = res
    return out.reshape(B, C, H, W)


# revision 18
# speedup vs baseline: 1.1917x; 1.0003x over previous
"""Trainium2 Bass kernel for nn_AttentionBlock (GroupNorm -> QKV -> 8-head
attention over T=1024 -> proj -> residual) on x[8, 512, 32, 32] f32.

Sharding: data-parallel over batch: core b handles sample b. No collectives.

v2 design (from NTFF trace analysis of v1 @198.7us):
  - ScalarE exp throughput governs the attention phase. v1 used 128 N=512
    ACTIVATEs (865ns each, ~40% fixed overhead). v2 iterates per (parity,
    st): scores land in one 2-bank [128,1024] PSUM tile -> ONE N=1024 exp
    per iteration (64 total), ping-ponged (bufs=2) so ScalarE never idles.
  - GroupNorm stats via DVE tensor_reduce (sum x) + ScalarE Square with
    accum_out (sum x^2): kills all 16 N=512 indicator stats matmuls;
    group-reduce is 4 tiny N=2 matmuls.
  - No zeroing open/close matmuls: each PSUM region opens with its own
    start=True (per-element has_written semantics, verified on HW).
  - No bias rank-1 matmuls: v-bias folded into proj bias on host
    (bp' = bp + wp @ bv, exact since softmax rows sum to 1); proj bias +
    residual fused into one scalar_tensor_tensor eviction.
  - h accumulates into one 2-bank [128,1024] tile; evicted RAW (copy) so
    the banks free immediately; softmax division (x inv broadcast via
    DRAM roundtrip DMA) applied later as background DVE work.
  - vt/qk(next pair) emission interleaved into the attention loop as
    background matmuls so TensorE stays dense behind the exp pipeline.

All matmul operands bf16 (except tiny f32 GN stat/broadcast matmuls);
PSUM accumulation f32. PSUM budget: score 2x[128,1024] (4 banks) +
hc [128,1024] (2) + sums [128,512] (1) + spare [128,512] (1) = 8 banks.
"""

import numpy as np

import concourse.bacc as bacc
import concourse.bass as bass
import concourse.mybir as mybir
import concourse.tile as tile
from concourse.bass_utils import run_bass_kernel_spmd

F32 = mybir.dt.float32
BF16 = mybir.dt.bfloat16
AF = mybir.ActivationFunctionType
ALU = mybir.AluOpType

B, C, H, W = 8, 512, 32, 32
T = H * W  # 1024
NH = 8  # heads
CH = C // NH  # 64 head channels
GROUPS = 32
GSIZE = C // GROUPS  # 16 channels per group
EPS = 1e-5
NCT = C // 128  # 4 channel tiles
NST = T // 128  # 8 spatial tiles
NPAIR = NH // 2  # 4 head pairs
CS = [slice(0, 512), slice(512, 1024)]


def _bcast_ap(src, n):
    """Partition-broadcast AP: replicate src's single partition n times."""
    ap = [[0, n]] + [list(d) for d in list(src.ap)[1:]]
    return bass.AP(tensor=src.tensor, offset=src.offset, ap=ap)


def build_nc(debug_taps=False):
    nc = bacc.Bacc(
        "TRN2",
        target_bir_lowering=False,
        debug=False,
        enable_asserts=False,
        num_devices=8,
    )

    x_d = nc.dram_tensor("x", [C, T], F32, kind="ExternalInput")
    wqk_d = nc.dram_tensor("wqk", [C, 2 * C], BF16, kind="ExternalInput")
    wv_d = nc.dram_tensor("wv", [C, C], BF16, kind="ExternalInput")
    wp_d = nc.dram_tensor("wp", [C, C], BF16, kind="ExternalInput")
    # all small params packed into one [128, 664] f32 blob (single DMA):
    # cols 0:128 ind (tile i at 32i), 128:640 indt (rows 0:32, tile i at
    # 128+128i), 640:644 gs, 644:648 gb, 648:652 bp2, 652:660 bqk.
    pb_d = nc.dram_tensor("pblob", [128, 660], F32, kind="ExternalInput")
    out_d = nc.dram_tensor("out", [C, T], F32, kind="ExternalOutput")
    dbg = {}
    if debug_taps:
        dbg["xn"] = nc.dram_tensor("dbg_xn", [NCT, 128, T], BF16, kind="ExternalOutput")
        dbg["qk"] = nc.dram_tensor("dbg_qk", [8, 128, T], BF16, kind="ExternalOutput")
        dbg["vt"] = nc.dram_tensor("dbg_vt", [NST, 128, C], BF16, kind="ExternalOutput")
        dbg["es"] = nc.dram_tensor("dbg_es", [2, 128, T], BF16, kind="ExternalOutput")
        dbg["sums"] = nc.dram_tensor("dbg_sums", [128, 512], F32, kind="ExternalOutput")
        dbg["ha"] = nc.dram_tensor("dbg_ha", [NCT, 128, T], BF16, kind="ExternalOutput")

    with tile.TileContext(nc) as tc:
        with (
            tc.tile_pool(name="sb", bufs=1) as sb,
            tc.tile_pool(name="ps", bufs=1, space="PSUM") as ps,
        ):
            # ---- input loads, split across engine DMA queues ----------------
            # sync: x0,x1 + param blob; scalar: x2,x3; gpsimd: all weights.
            xs, xns, wqks, wvs, wps = [], [], [], [], []
            for i in range(NCT):
                xt = sb.tile([128, T], F32, tag="x", bufs=NCT, name=f"x{i}")
                eng = nc.sync if i < 2 else nc.scalar
                eng.dma_start(out=xt, in_=x_d.ap()[128 * i : 128 * (i + 1), :])
                xs.append(xt)
            pb = sb.tile([128, 660], F32, tag="pblob", name="pb")
            nc.sync.dma_start(out=pb, in_=pb_d.ap())
            inds = [pb[:, 32 * i : 32 * (i + 1)] for i in range(NCT)]
            indts = [pb[0:GROUPS, 128 + 128 * i : 256 + 128 * i] for i in range(NCT)]
            gss = [pb[:, 640 + i : 641 + i] for i in range(NCT)]
            gbs = [pb[:, 644 + i : 645 + i] for i in range(NCT)]
            bp2s = [pb[:, 648 + i : 649 + i] for i in range(NCT)]
            bqks = [pb[:, 652 + mt : 653 + mt] for mt in range(8)]
            for i in range(NCT):
                wq = sb.tile([128, 2 * C], BF16, tag="wqk", bufs=NCT, name=f"wqk{i}")
                nc.gpsimd.dma_start(out=wq, in_=wqk_d.ap()[128 * i : 128 * (i + 1), :])
                wqks.append(wq)
            for i in range(NCT):
                wv = sb.tile([128, C], BF16, tag="wv", bufs=NCT, name=f"wv{i}")
                nc.gpsimd.dma_start(out=wv, in_=wv_d.ap()[128 * i : 128 * (i + 1), :])
                wvs.append(wv)
            for i in range(NCT):
                wp = sb.tile([128, C], BF16, tag="wp", bufs=NCT, name=f"wp{i}")
                nc.gpsimd.dma_start(out=wp, in_=wp_d.ap()[128 * i : 128 * (i + 1), :])
                wps.append(wp)

            ones_col = sb.tile([128, 1], BF16, tag="ones_c", name="ones_col")
            nc.vector.memset(ones_col, 1.0)

            # ---- HAM warmup: full-array matmul burst while DMAs land --------
            # PE_HAM unthrottles (1.2 -> 2.4 GHz) only after ~3.4us of
            # sustained REAL PE activity (rank-1 matmuls don't register);
            # burn ~16 full matmuls on a never-read accumulator so the head
            # matmuls run at full clock.
            wmA = sb.tile([128, 128], BF16, tag="wmA", name="wmA")
            nc.vector.memset(wmA, 0.001)
            wmB = sb.tile([128, 512], BF16, tag="wmB", name="wmB")
            nc.vector.memset(wmB, 0.001)
            wu = ps.tile([128, 512], F32, tag="spare", name="wu")
            for j in range(16):
                nc.tensor.matmul(
                    out=wu, lhsT=wmA, rhs=wmB,
                    start=(j == 0), stop=(j == 15),
                )

            # ---- GroupNorm statistics ---------------------------------------
            # per channel: sum_t x (DVE reduce) and sum_t x^2 (ScalarE Square
            # with accum_out); group-reduce both via one tiny N=2 matmul/tile.
            gsum = ps.tile([GROUPS, 2], F32, tag="spare", name="gsum")
            sx12s = []
            for i in range(NCT):
                sx12 = sb.tile([128, 2], F32, tag="sx12", bufs=NCT, name=f"sx12_{i}")
                nc.vector.tensor_reduce(
                    out=sx12[:, 0:1], in_=xs[i], axis=mybir.AxisListType.X, op=ALU.add
                )
                sqscr = sb.tile([128, T], BF16, tag="sqscr", bufs=2, name=f"sqscr{i}")
                nc.scalar.activation(
                    out=sqscr, in_=xs[i], func=AF.Square, accum_out=sx12[:, 1:2]
                )
                sx12s.append(sx12)
            for i in range(NCT):
                nc.tensor.matmul(
                    out=gsum, lhsT=inds[i], rhs=sx12s[i],
                    start=(i == 0), stop=(i == NCT - 1),
                )

            inv_n = 1.0 / (GSIZE * T)
            mr32 = sb.tile([GROUPS, 2], F32, tag="gnsm", bufs=8, name="mr32")
            nc.vector.tensor_scalar_mul(out=mr32, in0=gsum, scalar1=inv_n)
            msq = sb.tile([GROUPS, 1], F32, tag="gnsm", bufs=8, name="msq")
            nc.vector.tensor_mul(msq, mr32[:, 0:1], mr32[:, 0:1])
            var = sb.tile([GROUPS, 1], F32, tag="gnsm", bufs=8, name="var")
            nc.vector.tensor_sub(var, mr32[:, 1:2], msq)
            eps_t = sb.tile([GROUPS, 1], F32, tag="gnsm", bufs=8, name="eps_t")
            nc.vector.memset(eps_t, EPS)
            lnv = sb.tile([GROUPS, 1], F32, tag="gnsm", bufs=8, name="lnv")
            nc.scalar.activation(out=lnv, in_=var, func=AF.Ln, bias=eps_t, scale=1.0)
            nc.scalar.activation(out=mr32[:, 1:2], in_=lnv, func=AF.Exp, scale=-0.5)

            for i in range(NCT):
                mrb = ps.tile([128, 2], F32, tag="spare", name=f"mrb{i}")
                nc.tensor.matmul(out=mrb, lhsT=indts[i], rhs=mr32, start=True, stop=True)
                a_t = sb.tile([128, 1], F32, tag="gA", bufs=NCT, name=f"gA{i}")
                nc.vector.tensor_mul(a_t, mrb[:, 1:2], gss[i])
                tmp = sb.tile([128, 1], F32, tag="gT", bufs=2, name=f"gT{i}")
                nc.vector.tensor_mul(tmp, mrb[:, 0:1], a_t)
                b_t = sb.tile([128, 1], F32, tag="gB", bufs=NCT, name=f"gB{i}")
                nc.vector.tensor_sub(b_t, gbs[i], tmp)
                xn = sb.tile([128, T], BF16, tag="xn", bufs=NCT, name=f"xn{i}")
                nc.vector.tensor_scalar(
                    out=xn, in0=xs[i], scalar1=a_t, scalar2=b_t,
                    op0=ALU.mult, op1=ALU.add,
                )
                xns.append(xn)
                if debug_taps:
                    nc.sync.dma_start(out=dbg["xn"].ap()[i], in_=xn)

            # ---- QKV / V emission helpers -----------------------------------
            qks = [None] * 8
            vts = [None] * NST

            def emit_qk_half(mt, n, tag="spare"):
                """One t-chunk of q/k m-tile mt -> qks[mt][:, CS[n]]."""
                if qks[mt] is None:
                    qks[mt] = sb.tile([128, T], BF16, tag="qk", bufs=8, name=f"qk{mt}")
                qp = ps.tile(
                    [128, 512], F32, tag=tag,
                    bufs=(2 if tag == "sc" else None), name=f"qp{mt}_{n}",
                )
                for i in range(NCT):
                    nc.tensor.matmul(
                        out=qp,
                        lhsT=wqks[i][:, 128 * mt : 128 * (mt + 1)],
                        rhs=xns[i][:, CS[n]],
                        start=(i == 0),
                        stop=(i == NCT - 1),
                    )
                nc.vector.tensor_scalar_add(
                    out=qks[mt][:, CS[n]], in0=qp, scalar1=bqks[mt]
                )
                if debug_taps and n == 1:
                    nc.sync.dma_start(out=dbg["qk"].ap()[mt], in_=qks[mt])

            def emit_vt_part(st, ilo, ihi, tag="spare"):
                vp = ps.tile(
                    [128, 512], F32, tag=tag,
                    bufs=(2 if tag == "sc" else None), name=f"vp{st}",
                ) if ilo == 0 else emit_vt_part.vp
                emit_vt_part.vp = vp
                for i in range(ilo, ihi):
                    nc.tensor.matmul(
                        out=vp,
                        lhsT=xns[i][:, 128 * st : 128 * (st + 1)],
                        rhs=wvs[i],
                        start=(i == 0),
                        stop=(i == NCT - 1),
                    )
                if ihi == NCT:
                    vt = sb.tile([128, C], BF16, tag="vt", bufs=NST, name=f"vt{st}")
                    nc.vector.tensor_copy(vt, vp)
                    vts[st] = vt
                    if debug_taps:
                        nc.sync.dma_start(out=dbg["vt"].ap()[st], in_=vt)

            def emit_vt(st, tag="spare"):
                emit_vt_part(st, 0, NCT, tag=tag)

            def emit_qk_part(mt, n, ilo, ihi, tag="spare"):
                """K-subrange [ilo,ihi) of one qk half; evict when ihi==NCT."""
                qp = ps.tile(
                    [128, 512], F32, tag=tag,
                    bufs=(2 if tag == "sc" else None), name=f"qp{mt}_{n}",
                ) if ilo == 0 else emit_qk_part.qp
                emit_qk_part.qp = qp
                for i in range(ilo, ihi):
                    nc.tensor.matmul(
                        out=qp,
                        lhsT=wqks[i][:, 128 * mt : 128 * (mt + 1)],
                        rhs=xns[i][:, CS[n]],
                        start=(i == 0),
                        stop=(i == NCT - 1),
                    )
                if ihi == NCT:
                    nc.vector.tensor_scalar_add(
                        out=qks[mt][:, CS[n]], in0=qp, scalar1=bqks[mt]
                    )

            # prologue: q/k for pair 0 and vt 0-3 through the (still idle)
            # score-pool banks; vt matmuls fill PE gaps left by qk evictions.
            emit_qk_half(0, 0, tag="sc")
            emit_vt(1, tag="sc")
            emit_qk_half(0, 1, tag="sc")
            emit_vt(2, tag="sc")
            emit_qk_half(1, 0, tag="sc")
            emit_vt(3, tag="sc")
            emit_qk_half(1, 1, tag="sc")
            emit_vt(0, tag="sc")

            # background thunks, one popped per attention slot
            order = []
            for st in range(4, NST):
                order.append(lambda st=st: emit_vt(st))
            for mt in range(2, 8):
                qks[mt] = sb.tile([128, T], BF16, tag="qk", bufs=8, name=f"qk{mt}")
                for n in range(2):
                    order.append(lambda mt=mt, n=n: emit_qk_part(mt, n, 0, 2))
                    order.append(lambda mt=mt, n=n: emit_qk_part(mt, n, 2, 4))

            # ---- attention: flat 64-slot pipeline across all pairs ----------
            has, hcs, sums_l, pair_objs = [], [], [], []
            scs, es = {}, {}

            def pair_state(p):
                hc = ps.tile([128, T], F32, tag="hc", name=f"hc{p}")
                sums = ps.tile([128, 512], F32, tag="sums", name=f"sums{p}")
                hcs.append(hc)
                sums_l.append(sums)

            def sc_mms(K):
                p, k = K // 16, K % 16
                par, st = k // 8, k % 8
                qtile, ktile = qks[2 * p], qks[2 * p + 1]
                pr = slice(64 * par, 64 * (par + 1))
                ss = slice(128 * st, 128 * (st + 1))
                sc = ps.tile([128, T], F32, tag="sc", bufs=2, name=f"sc{K}")
                for n in range(2):
                    nc.tensor.matmul(
                        out=sc[:, CS[n]], lhsT=ktile[pr, ss],
                        rhs=qtile[pr, CS[n]], start=True, stop=True,
                    )
                scs[K] = sc

            def exp_act(K):
                e = sb.tile([128, T], BF16, tag="es", bufs=3, name=f"e{K}")
                nc.scalar.activation(out=e, in_=scs[K], func=AF.Exp, scale=0.125)
                es[K] = e
                if debug_taps and K in (0, 8):
                    nc.sync.dma_start(out=dbg["es"].ap()[K // 8], in_=e)

            def h_sums_mms(K):
                p, k = K // 16, K % 16
                par, st = k // 8, k % 8
                e = es.pop(K)
                hc, sums = hcs[p], sums_l[p]
                first, last = st == 0, st == NST - 1
                vsl = slice(128 * p + 64 * par, 128 * p + 64 * (par + 1))
                for n in range(2):
                    nc.tensor.matmul(
                        out=hc[64 * par : 64 * (par + 1), CS[n]],
                        lhsT=vts[st][:, vsl], rhs=e[:, CS[n]],
                        start=first, stop=last, skip_group_check=True,
                    )
                for n in range(2):
                    r = 64 * par + 32 * n
                    nc.tensor.matmul(
                        out=sums[r : r + 1, :], lhsT=ones_col, rhs=e[:, CS[n]],
                        start=first, stop=last, skip_group_check=True,
                        tile_position=(0, r),
                    )

            def pair_tail(p):
                """Emit right after h_sums of slot 16p+15 (cross-pair sc/exp
                for p+1 already issued). DRAM-roundtrip partition broadcast
                of 1/sums; raw hc eviction frees the banks; inv multiply is
                deferred to background."""
                invs = sb.tile([128, 512], F32, tag="invs", bufs=2, name=f"invs{p}")
                nc.vector.reciprocal_approx_fast(out=invs, in_=sums_l[p])
                if debug_taps and p == 0:
                    nc.sync.dma_start(out=dbg["sums"].ap(), in_=invs)
                if p < NPAIR - 1:
                    # evict raw h so the hc banks free for the next pair
                    har = sb.tile([128, T], F32, tag="har", bufs=2, name=f"har{p}")
                    nc.vector.tensor_copy(har, hcs[p])
                else:
                    har = hcs[p]  # last pair: multiply straight out of PSUM
                ha = sb.tile([128, T], BF16, tag="ha", bufs=NPAIR, name=f"ha{p}")
                has.append(ha)
                inv = sb.tile([128, T], F32, tag="inv", bufs=2, name=f"inv{p}")
                invd = nc.dram_tensor(f"invd{p}", [4, 512], F32)
                for j, row in enumerate((0, 32, 64, 96)):
                    nc.sync.dma_start(
                        out=invd.ap()[j : j + 1, :], in_=invs[row : row + 1, :]
                    )
                for j, (dp, n) in enumerate(((0, 0), (0, 1), (64, 0), (64, 1))):
                    nc.sync.dma_start(
                        out=inv[dp : dp + 64, CS[n]],
                        in_=_bcast_ap(invd.ap()[j : j + 1, :], 64),
                    )

                def ha_mul(ha=ha, har=har, inv=inv, p=p):
                    nc.vector.tensor_mul(ha, har, inv)
                    if debug_taps:
                        nc.sync.dma_start(out=dbg["ha"].ap()[p], in_=ha)

                if p < NPAIR - 1:
                    order.append(ha_mul)
                else:
                    pair_objs.append(ha_mul)

            # pipeline: sc two ahead, exp one ahead, h/sums at K, 1 bg thunk
            pair_state(0)
            sc_mms(0)
            sc_mms(1)
            exp_act(0)
            for K in range(64):
                if K % 16 == 15 and K // 16 < 3:
                    pair_state(K // 16 + 1)
                if K + 2 < 64:
                    sc_mms(K + 2)
                if K + 1 < 64:
                    exp_act(K + 1)
                h_sums_mms(K)
                if K % 16 == 15:
                    pair_tail(K // 16)
                if order:
                    order.pop(0)()

            while order:
                order.pop(0)()

            # ---- proj + residual (pair-3 K-tiles last: its ha lands late) ---
            pps = {}

            def start_proj(mt, tag):
                pp = ps.tile(
                    [128, T], F32, tag=tag,
                    bufs=(2 if tag == "sc" else None), name=f"pp{mt}",
                )
                pps[mt] = pp
                for i in range(NCT - 1):
                    for n in range(2):
                        nc.tensor.matmul(
                            out=pp[:, CS[n]],
                            lhsT=wps[i][:, 128 * mt : 128 * (mt + 1)],
                            rhs=has[i][:, CS[n]],
                            start=(i == 0),
                            stop=False,
                        )

            def finish_proj(mt):
                pp = pps[mt]
                for n in range(2):
                    nc.tensor.matmul(
                        out=pp[:, CS[n]],
                        lhsT=wps[3][:, 128 * mt : 128 * (mt + 1)],
                        rhs=has[3][:, CS[n]],
                        start=False,
                        stop=True,
                    )
                ot = sb.tile([128, T], F32, tag="ot", bufs=2, name=f"ot{mt}")
                nc.vector.scalar_tensor_tensor(
                    out=ot, in0=pp, scalar=bp2s[mt], in1=xs[mt],
                    op0=ALU.add, op1=ALU.add,
                )
                nc.sync.dma_start(
                    out=out_d.ap()[128 * mt : 128 * (mt + 1), :], in_=ot
                )

            # pairs 0-2 accumulate while pair-3's inv multiply is in flight
            start_proj(0, "sc")
            start_proj(1, "sc")
            pair_objs[0]()  # ha_mul3 on DVE (reads hc3 PSUM directly)
            finish_proj(0)
            finish_proj(1)
            start_proj(2, "hc")
            finish_proj(2)
            start_proj(3, "sc")
            finish_proj(3)

    nc.compile()
    return nc


def prep_inputs(x, gn_scale, gn_bias, qkv_w, qkv_b, proj_w, proj_b):
    """Host-side rearrangement into the per-core input map (shared across cores
    except x)."""
    x = np.asarray(x, dtype=np.float32)
    qkv_w = np.asarray(qkv_w, dtype=np.float32)
    qkv_b = np.asarray(qkv_b, dtype=np.float32)
    proj_w = np.asarray(proj_w, dtype=np.float32)
    proj_b = np.asarray(proj_b, dtype=np.float32)
    gn_scale = np.asarray(gn_scale, dtype=np.float32)
    gn_bias = np.asarray(gn_bias, dtype=np.float32)

    wq3 = qkv_w.reshape(NH, 3 * CH, C)  # per head: [q(64); k(64); v(64)] rows
    q_rows = wq3[:, 0:CH, :]  # [8, 64, 512]
    k_rows = wq3[:, CH : 2 * CH, :]
    v_rows = wq3[:, 2 * CH : 3 * CH, :]
    b3 = qkv_b.reshape(NH, 3 * CH)
    qb, kb, vb = b3[:, 0:CH], b3[:, CH : 2 * CH], b3[:, 2 * CH : 3 * CH]

    # wqk columns: per pair p: [q_2p | q_2p+1 | k_2p | k_2p+1] (128+128)
    cols = []
    bqk = []
    for p in range(NPAIR):
        cols += [q_rows[2 * p], q_rows[2 * p + 1], k_rows[2 * p], k_rows[2 * p + 1]]
        bqk += [qb[2 * p], qb[2 * p + 1], kb[2 * p], kb[2 * p + 1]]
    wqk = np.concatenate(cols, axis=0).T.copy()  # [512, 1024]
    bqk = np.concatenate(bqk).reshape(8, 128)

    wv = v_rows.reshape(C, C).T.copy()  # [512, 512] (c, c'-head-major)
    wp = proj_w.T.copy()
    # v-bias folded into proj bias: softmax rows sum to 1, so
    # h_withbias = h + bv  =>  proj(h) + proj_w @ bv + proj_b.
    vb_hm = vb.reshape(C)  # head-major v bias, matches proj_w columns
    bp2 = (proj_b + proj_w @ vb_hm).reshape(NCT, 128)

    ind = np.zeros((NCT, 128, GROUPS), dtype=np.float32)
    for i in range(NCT):
        for cl in range(128):
            ind[i, cl, 8 * i + cl // GSIZE] = 1.0
    indt = np.ascontiguousarray(ind.transpose(0, 2, 1))

    pblob = np.zeros((128, 660), dtype=np.float32)
    for i in range(NCT):
        pblob[:, 32 * i : 32 * (i + 1)] = ind[i]
        pblob[0:GROUPS, 128 + 128 * i : 256 + 128 * i] = indt[i]
        pblob[:, 640 + i] = gn_scale[128 * i : 128 * (i + 1)]
        pblob[:, 644 + i] = gn_bias[128 * i : 128 * (i + 1)]
        pblob[:, 648 + i] = bp2[i]
    for mt in range(8):
        pblob[:, 652 + mt] = bqk[mt]

    import ml_dtypes

    bf16 = ml_dtypes.bfloat16
    shared = {
        "wqk": wqk.astype(bf16), "wv": wv.astype(bf16), "wp": wp.astype(bf16),
        "pblob": pblob,
    }
    in_maps = []
    for b in range(B):
        m = dict(shared)
        m["x"] = np.ascontiguousarray(x[b].reshape(C, T))
        in_maps.append(m)
    return in_maps


_NC_CACHE = {}


def _get_nc():
    if "nc" not in _NC_CACHE:
        _NC_CACHE["nc"] = build_nc()
    return _NC_CACHE["nc"]


def kernel(x, gn_scale, gn_bias, qkv_w, qkv_b, proj_w, proj_b, **run_kwargs):
    nc = _get_nc()
    in_maps = prep_inputs(x, gn_scale, gn_bias, qkv_w, qkv_b, proj_w, proj_b)
    res = run_bass_kernel_spmd(nc, in_maps, core_ids=list(range(B)), **run_kwargs)
    out = np.stack([res.results[b]["out"] for b in range(B)])
    kernel.last_results = res
    return out.reshape(B, C, H, W)
